# revision 4
# baseline (speedup 1.0000x reference)
"""Trainium2 Bass/Tile kernel for nn_LogEntmaxBisect (log(entmax_bisect(X, alpha=1.5))).

Math: the reference bisects for the entmax threshold tau per row, then returns
log(p / sum(p)) with p = relu(X*(alpha-1) - tau)^2.  For the target input
distribution the entmax support is dense on every row (all p_i > 0), so tau has
a closed form: with Xs = 0.5*X, S1 = sum(Xs), S2 = sum(Xs^2) per row,

    sum_i (Xs_i - tau)^2 = 1  =>  d*tau^2 - 2*S1*tau + (S2 - 1) = 0
    tau = (S1 - sqrt(S1^2 - d*(S2 - 1))) / d        (root below the data)

and the output is 2*ln(Xs_i - tau) - ln(Z), Z = 1 + (r^2 - disc)/d ~= 1.

If the support is NOT dense, excluded elements produce ln(<=0) = NaN/-inf on
device; kernel() detects that and falls back to an exact numpy bisection.

Layout per core (rows sharded 8 ways -> 512 rows/core):
  4 row-tiles of 128 partitions; 16 column-chunks of 2000 (1 MB DMAs).
  Stats: one DVE pass (bn_stats subgroups of 500 -> bn_aggr) -> mean/var.
  Finalize ([128,1] vectors): sqrt via exp(0.5*ln(disc)) + 1 Newton step
  (Ln/Exp/Square all live in the natural_log_exp_and_others ACT table set, so
  there is a single table load in the whole kernel).
  Output: ACT Ln(0.5*x + (-tau)) per chunk, then DVE (*2 - lnZ), DMA out.
  x chunks stay resident in SBUF between the two passes: DMA = 64+64 MB/core.
"""

import numpy as np

R, D = 4096, 32000
N_CORES = 8
RPC = R // N_CORES  # 512 rows per core
P = 128
NRT = RPC // P      # 4 row-tiles per core
CHUNK = 2000
NCH = D // CHUNK    # 16
SUB = 500           # bn_stats subgroup size (<= 512, divides CHUNK)
NSUB = CHUNK // SUB

_compiled = {}


def _build_nc():
    import concourse.bacc as bacc
    import concourse.tile as tile
    from concourse import mybir

    f32 = mybir.dt.float32
    AF = mybir.ActivationFunctionType
    OP = mybir.AluOpType

    nc = bacc.Bacc()
    x = nc.dram_tensor("x", [RPC, D], f32, kind="ExternalInput")
    y = nc.dram_tensor("y", [RPC, D], f32, kind="ExternalOutput")

    with tile.TileContext(nc) as tc:
        with (
            tc.tile_pool(name="xp", bufs=NCH + 2) as xp,
            tc.tile_pool(name="op", bufs=4) as op,
            tc.tile_pool(name="sp", bufs=2) as sp,
            tc.tile_pool(name="fin", bufs=2) as fin,
        ):
            for rt in range(NRT):
                r0, r1 = rt * P, (rt + 1) * P
                stats = sp.tile([P, NCH * NSUB, 6], f32)
                xcs = []
                for c in range(NCH):
                    xc = xp.tile([P, CHUNK], f32, tag="xc")
                    nc.sync.dma_start(
                        out=xc, in_=x[r0:r1, c * CHUNK:(c + 1) * CHUNK]
                    )
                    for s in range(NSUB):
                        nc.vector.bn_stats(
                            out=stats[:, c * NSUB + s, :],
                            in_=xc[:, s * SUB:(s + 1) * SUB],
                        )
                    xcs.append(xc)

                mv = fin.tile([P, 2], f32, tag="mv")
                nc.vector.bn_aggr(out=mv, in_=stats)

                # S1 = 0.5*d*mean ; S2 = 0.25*d*(var + mean^2)   (moments of Xs=0.5*x)
                s1 = fin.tile([P, 1], f32, tag="s1")
                nc.vector.tensor_scalar_mul(s1, mv[:, 0:1], float(0.5 * D))
                msq = fin.tile([P, 1], f32, tag="msq")
                nc.vector.tensor_mul(msq, mv[:, 0:1], mv[:, 0:1])
                s2 = fin.tile([P, 1], f32, tag="s2")
                nc.vector.tensor_add(s2, mv[:, 1:2], msq)
                # disc = S1^2 + (-d*S2 + d)
                s1sq = fin.tile([P, 1], f32, tag="s1sq")
                nc.vector.tensor_mul(s1sq, s1, s1)
                t0 = fin.tile([P, 1], f32, tag="t0")
                nc.vector.tensor_scalar(
                    t0, s2, float(-0.25 * D * D), float(D), op0=OP.mult, op1=OP.add
                )
                disc = fin.tile([P, 1], f32, tag="disc")
                nc.vector.tensor_add(disc, s1sq, t0)

                # r0 = exp(0.5*ln(disc)) ~= sqrt(disc); one Newton step
                lnd = fin.tile([P, 1], f32, tag="lnd")
                nc.scalar.activation(lnd, disc, AF.Ln)
                sq0 = fin.tile([P, 1], f32, tag="sq0")
                nc.scalar.activation(sq0, lnd, AF.Exp, scale=0.5)
                rec = fin.tile([P, 1], f32, tag="rec")
                nc.vector.reciprocal(rec, sq0)
                q = fin.tile([P, 1], f32, tag="q")
                nc.vector.tensor_mul(q, disc, rec)
                rsum = fin.tile([P, 1], f32, tag="rsum")
                nc.vector.tensor_add(rsum, sq0, q)
                rr = fin.tile([P, 1], f32, tag="rr")
                nc.vector.tensor_scalar_mul(rr, rsum, 0.5)

                # negtau = (r - S1)/d ; lnZ = ln(1 + (r^2 - disc)/d)
                rmS1 = fin.tile([P, 1], f32, tag="rmS1")
                nc.vector.tensor_sub(rmS1, rr, s1)
                negtau = fin.tile([P, 1], f32, tag="negtau")
                nc.vector.tensor_scalar_mul(negtau, rmS1, float(1.0 / D))
                rrsq = fin.tile([P, 1], f32, tag="rrsq")
                nc.vector.tensor_mul(rrsq, rr, rr)
                zd = fin.tile([P, 1], f32, tag="zd")
                nc.vector.tensor_sub(zd, rrsq, disc)
                zz = fin.tile([P, 1], f32, tag="zz")
                nc.vector.tensor_scalar(
                    zz, zd, float(1.0 / D), 1.0, op0=OP.mult, op1=OP.add
                )
                lnz = fin.tile([P, 1], f32, tag="lnz")
                nc.scalar.activation(lnz, zz, AF.Ln)

                for c in range(NCH):
                    oc = op.tile([P, CHUNK], f32, tag="oc")
                    nc.scalar.activation(oc, xcs[c], AF.Ln, bias=negtau, scale=0.5)
                    nc.vector.tensor_scalar(
                        oc, oc, 2.0, lnz, op0=OP.mult, op1=OP.subtract
                    )
                    nc.sync.dma_start(
                        out=y[r0:r1, c * CHUNK:(c + 1) * CHUNK], in_=oc
                    )
    nc.finalize()
    return nc


def _numpy_fallback(X):
    """Exact replica of the reference bisection (only used if support is sparse)."""
    ALPHA, N_ITER = 1.5, 50
    Xs = (X * np.float32(ALPHA - 1.0)).astype(np.float32)
    d = Xs.shape[-1]
    max_val = Xs.max(axis=-1, keepdims=True)
    tau_lo = max_val - np.float32(1.0)
    tau_hi = max_val - np.float32((1.0 / d) ** (ALPHA - 1.0))

    def p(z):
        return np.square(np.maximum(z, np.float32(0.0)))

    f_lo = p(Xs - tau_lo).sum(axis=-1, keepdims=True) - np.float32(1.0)
    dm = tau_hi - tau_lo
    tau_m = tau_lo
    for _ in range(N_ITER):
        dm = dm * np.float32(0.5)
        tau_m = tau_lo + dm
        f_m = p(Xs - tau_m).sum(axis=-1, keepdims=True) - np.float32(1.0)
        tau_lo = np.where(f_m * f_lo >= 0, tau_m, tau_lo)
    pp = p(Xs - tau_m)
    pp = pp / pp.sum(axis=-1, keepdims=True)
    pp = pp / pp.sum(axis=-1, keepdims=True)
    return np.log(pp).astype(np.float32)


def kernel(**inputs):
    from concourse.bass_utils import run_bass_kernel_spmd

    X = np.ascontiguousarray(np.asarray(inputs["X"], dtype=np.float32))
    assert X.shape == (R, D), X.shape

    if "nc" not in _compiled:
        _compiled["nc"] = _build_nc()
    nc = _compiled["nc"]

    in_maps = [{"x": X[i * RPC:(i + 1) * RPC]} for i in range(N_CORES)]
    res = run_bass_kernel_spmd(nc, in_maps, list(range(N_CORES)))
    out = np.concatenate([res.results[i]["y"] for i in range(N_CORES)], axis=0)

    if not np.isfinite(out).all():
        # sparse entmax support (not expected for the target distribution)
        return _numpy_fallback(X)
    return out
